# revision 9
# baseline (speedup 1.0000x reference)
"""AcidSynth Trainium2 kernel (v4).

Only the first 8192 output samples are nonzero (env dies at t=6000; the
dissipative biquad state underflows to fp32 zero soon after). 8 cores
each compute a 4096-sample chunk (3072 warmup + 1024 payload at rows
96:128 of a [128 x 32] layout); the rest of the 524288-sample output is
assembled as zeros on host.

Per-sample affine state maps are HOMOGENEOUS 3x3 blocks (9 slots,
row-major, constant bottom row (0,0,1) written once at setup), so a map
compose Z = X o Y is plainly Z = X @ Y restricted to rows 0-1:
  * latency-critical composes (DVE) = 2 strided mults + one
    tensor_reduce over k — dependency depth 2, no separate d-fix;
  * throughput composes (Pool full-width ladder) = 2 mults (k in {0,1})
    + pair-add + d-fix, which is cheaper there.

Cross-row state uses the 16-row (512-sample) windowed composition with
the same accuracy envelope as the validated baseline. Key scheduling:
  * dummy Sin activation at the ACT queue head prefetches the 1283ns
    trig table during the input DMA;
  * env (pure function of scalar alpha and t) host-computed; oscillator
    (phase recurrence) on device;
  * a mini end-column ladder (E4/E8/E16: span-4/8/16 composites at the
    16/8/4... end columns only) races ahead on DVE, so the PE shift
    bursts + 16-row window tree run ~4us before the full ladder is done;
  * the full-width ladder: M4 on DVE inside the burst-1 PE latency gap,
    M8/M16 on Pool concurrently with the DVE window tree;
  * no identity fixups in shift bursts (only rows 96:128 are output);
  * apply is split: y[0:17] reads M16's row0 directly (prefix spans
    <= 16 need no extra compose; col PAD-1 is the identity pad), y[17:32]
    reads a 15-column row0-compose FR = row0(M16[16+j] o M16[j]).
"""

import numpy as np

R = 128          # rows (SBUF partitions)
L = 32           # samples per row
PAD = 16         # identity-map pad columns for in-row KS shifts
W = L + PAD
CH = R * L       # per-core chunk = 4096
PAY = 1024       # payload samples per core
A = 8192         # active window
N = 524288
SC = 8           # scalar columns in the input pack
IC = SC + 3 * L + 28   # input cols padded to 132 (528B rows: full-rate DMA)

_cache = {}


def _emit(nc, tc, pool, psum_pool, in_all, y_out):
    import concourse.mybir as mybir

    F = mybir.dt.float32
    I32 = mybir.dt.int32
    Alu = mybir.AluOpType
    Act = mybir.ActivationFunctionType
    Ax = mybir.AxisListType
    V = nc.vector
    S = nc.scalar
    GP = nc.gpsimd

    def T(name, shape, dtype=F):
        return pool.tile(shape, dtype, name=name, tag=name)

    # ---------------- input DMA (single, posted first) ----------------
    allin = T("allin", [R, IC])
    nc.sync.dma_start(out=allin, in_=in_all)
    sc = allin[:, 0:SC]
    wv = allin[:, SC:SC + L]
    qv = allin[:, SC + L:SC + 2 * L]
    env = allin[:, SC + 2 * L:SC + 3 * L]
    rosc_ap = sc[:, 0:1]
    pbase_ap = sc[:, 1:2]
    zi1_ap = sc[:, 2:3]
    zi2_ap = sc[:, 3:4]

    # ---------------- pre-DMA setup ----------------
    M2 = T("M2", [R, W * 9])
    M4 = T("M4", [R, W * 9])
    M8 = T("M8", [R, W * 9])
    M16 = T("M16", [R, W * 9])
    NAC = T("NAC", [R, L * 4])       # per-sample (na1, na2, c1, c2)
    KS1 = T("KS1", [R, 4 * 18])      # burst-1 shifted 18-col packs (SBUF)
    KS2 = T("KS2", [R, 4 * 9])       # burst-2 shifted K4 maps (SBUF)

    def m9(M):
        return M.rearrange("p (t x) -> p t x", x=9)

    bcs = T("bcs", [R, 2])
    SCLW = float(np.float32(2.0 * np.pi * 7900.0 / 48000.0))
    BS = float(np.float32(2.0 * np.pi * 100.0 / 48000.0))
    BC = float(np.float32(BS + np.pi / 2))
    V.memset(bcs[:, 0:1], BC)
    V.memset(bcs[:, 1:2], BS)
    # Dummy Sin with no DMA dependency: hoists the trig table load to the
    # ACT queue head so it overlaps the input DMA. Its output cell is
    # overwritten by the burst-1 PSUM copy before any read.
    S.activation(KS1[:, 0:1], bcs[:, 0:1], Act.Sin)

    # PAD identity maps + constant (0,0,1) bottom row on ALL W columns of
    # the map tiles that serve as Y operands of reduce-composes.
    for M in (M2, M4, M8, M16):
        V.memset(M[:, 0:PAD * 9], 0.0)
        V.memset(m9(M)[:, 0:PAD, 0:1], 1.0)
        V.memset(m9(M)[:, 0:PAD, 4:5], 1.0)
        V.memset(m9(M)[:, PAD:W, 6:8], 0.0)
        V.memset(m9(M)[:, 0:W, 8:9], 1.0)
        V.memset(m9(M)[:, PAD:W, 6:7], 0.0)
    V.memset(M2[:, PAD * 9 + 1:PAD * 9 + 2], 1.0)   # t=0: a01 = 1
    V.memset(M2[:, PAD * 9 + 4:PAD * 9 + 5], 0.0)   # t=0: a11 = 0

    def row2_const(Mt, G):
        v = Mt.rearrange("p (g x) -> p g x", x=9)
        V.memset(v[:, :, 6:8], 0.0)
        V.memset(v[:, :, 8:9], 1.0)

    E4 = T("E4", [R, 8 * 9])     # span-4 composites at t = 4j+3
    E8 = T("E8", [R, 4 * 9])     # span-8 at t = 8j+7
    E16 = T("E16", [R, 2 * 9])   # span-16 at t = 15, 31
    KH = T("KH", [R, 4 * 9])     # H, Hs1, Hs2, Hs3 (H = span-32 row map)
    TF = T("TF", [R, 2 * 9])
    K4 = T("K4", [R, 9])
    TT = T("TT", [R, 2 * 9])
    K16 = T("K16", [R, 9])
    for Mt, G in ((E4, 8), (E8, 4), (E16, 2), (KH, 4), (TF, 2), (K4, 1),
                  (TT, 2), (K16, 1)):
        row2_const(Mt, G)

    ji = T("ji", [R, L], I32)
    GP.iota(ji, pattern=[[1, L]], base=0, channel_multiplier=0)
    jf = T("jf", [R, L])
    V.tensor_copy(out=jf, in_=ji)
    ii = T("ii", [R, R], I32)        # ii[c, j] = j - c
    GP.iota(ii, pattern=[[1, R]], base=0, channel_multiplier=-1)
    iif = T("iif", [R, R])
    V.tensor_copy(out=iif, in_=ii)
    sh = {}
    for n, eng in ((0, V), (1, V), (2, V), (3, V), (5, GP), (9, GP), (13, GP)):
        m = T("sh%d" % n, [R, R])
        eng.tensor_scalar(m, iif, float(n), None, Alu.is_equal)
        sh[n] = m

    # ---------------- coefficient chain (post-DMA) ----------------
    cw = T("cw", [R, L])
    S.activation(cw, wv, Act.Sin, bias=bcs[:, 0:1], scale=SCLW)
    sw = T("sw", [R, L])
    S.activation(sw, wv, Act.Sin, bias=bcs[:, 1:2], scale=SCLW)
    q2 = T("q2", [R, L])
    V.tensor_scalar(q2, qv, float(np.float32(2.0 * (8.0 - 0.7071))),
                    float(np.float32(2.0 * 0.7071)), Alu.mult, Alu.add)
    rq = T("rq", [R, L])
    V.reciprocal(rq, q2)
    # oscillator (independent of the w/q chain)
    uph = T("uph", [R, L])
    V.tensor_scalar(uph, jf, rosc_ap, pbase_ap, Alu.mult, Alu.add)
    ge1 = T("ge1", [R, L])
    V.tensor_scalar(ge1, uph, 1.0, None, Alu.is_ge)
    ph = T("ph", [R, L])
    V.tensor_tensor(out=ph, in0=uph, in1=ge1, op=Alu.subtract)
    dp = T("dp", [R, L])
    V.tensor_scalar(dp, ph, 0.5, 0.5, Alu.is_lt, Alu.subtract)
    dry = T("dry", [R, L])
    V.tensor_mul(dry, dp, env)

    af = T("af", [R, L])
    V.tensor_mul(af, sw, rq)
    a0 = T("a0", [R, L])
    V.tensor_scalar_add(a0, af, 1.0)
    r0 = T("r0", [R, L])
    V.reciprocal(r0, a0)
    cwh = T("cwh", [R, L])           # (1-cw)/2
    V.tensor_scalar(cwh, cw, -0.5, 0.5, Alu.mult, Alu.add)
    cd = T("cd", [R, L])             # (1-cw)/2 * dry
    V.tensor_mul(cd, cwh, dry)

    NAC4 = NAC.rearrange("p (t s) -> p t s", s=4)
    na1v = NAC4[:, :, 0:1].squeeze(2)
    na2v = NAC4[:, :, 1:2].squeeze(2)
    c1v = NAC4[:, :, 2:3].squeeze(2)
    c2v = NAC4[:, :, 3:4].squeeze(2)
    V.scalar_tensor_tensor(out=na1v, in0=cw, scalar=2.0, in1=r0,
                           op0=Alu.mult, op1=Alu.mult)
    # na2 = (af-1)/a0 = 1 - 2*r0
    V.tensor_scalar(na2v, r0, -2.0, 1.0, Alu.mult, Alu.add)
    b0d = T("b0d", [R, L])           # b0*dry
    V.tensor_mul(b0d, cd, r0)
    V.scalar_tensor_tensor(out=c1v, in0=na1v, scalar=2.0, in1=b0d,
                           op0=Alu.add, op1=Alu.mult)
    V.scalar_tensor_tensor(out=c2v, in0=na2v, scalar=1.0, in1=b0d,
                           op0=Alu.add, op1=Alu.mult)

    # ---------------- span-2 construct into M2 ----------------
    # Z[t]: a00 = na1_t*na1' + na2';  a01 = na1_t
    #       d1  = na1_t*c1'  + c2' + c1_t
    #       a10 = na2_t*na1';         a11 = na2_t
    #       d2  = na2_t*c1'  + c2_t           (x' = x_{t-1})
    M2trg = M2.rearrange("p (t r g) -> p t r g", r=3, g=3)
    Lm = L - 1
    GP.tensor_copy(out=M2trg[:, PAD + 1:W, 0:2, 1:2].squeeze(3),
                   in_=NAC4[:, 1:L, 0:2])
    GP.tensor_copy(out=M2trg[:, PAD:PAD + 1, 0:2, 0:1].squeeze(3).squeeze(1),
                   in_=NAC4[:, 0:1, 0:2].squeeze(1))
    GP.tensor_copy(out=M2trg[:, PAD:PAD + 1, 0:2, 2:3].squeeze(3).squeeze(1),
                   in_=NAC4[:, 0:1, 2:4].squeeze(1))
    pm_out = M2trg[:, PAD + 1:W, 0:2, 0:3:2]
    V.tensor_tensor(
        out=pm_out,
        in0=NAC4[:, 1:L, 0:2].unsqueeze(3).broadcast_to((R, Lm, 2, 2)),
        in1=NAC4[:, 0:Lm, 0:3:2].unsqueeze(2).broadcast_to((R, Lm, 2, 2)),
        op=Alu.mult)
    aa_out = M2trg[:, PAD + 1:W, 0:1, 0:3:2].squeeze(2)   # {a00, d1}
    V.tensor_tensor(out=aa_out, in0=aa_out, in1=NAC4[:, 0:Lm, 1:4:2],
                    op=Alu.add)
    ab_out = M2trg[:, PAD + 1:W, 0:2, 2:3].squeeze(3)     # {d1, d2}
    V.tensor_tensor(out=ab_out, in0=ab_out, in1=NAC4[:, 1:L, 2:4],
                    op=Alu.add)

    # ---------------- composes ----------------
    def compose_full(eng, OUT, IN, d, PPt):
        """OUT[t] = IN[t] o IN[t-d], all columns (2 mults + add + fix)."""
        PPv = PPt.rearrange("p (r t i k) -> p r t i k", r=2, t=L, i=3, k=2)
        INx = m9(IN)
        Yv = (IN.rearrange("p (t k i) -> p t k i", k=3, i=3)
              [:, PAD - d:W - d, 0:2].rearrange("p t k i -> p t i k"))
        for r in (0, 1):
            Xr = (INx[:, PAD:W, 3 * r:3 * r + 2]
                  .unsqueeze(2).broadcast_to((R, L, 3, 2)))
            eng.tensor_tensor(out=PPv[:, r], in0=Xr, in1=Yv, op=Alu.mult)
        OUTtrg = OUT.rearrange("p (t r g) -> p t r g", r=3, g=3)
        PPtr = PPt.rearrange("p (r t i k) -> p t r i k", r=2, t=L, i=3, k=2)
        eng.tensor_tensor(out=OUTtrg[:, PAD:W, 0:2], in0=PPtr[:, :, :, :, 0],
                          in1=PPtr[:, :, :, :, 1], op=Alu.add)
        dout = OUTtrg[:, PAD:W, 0:2, 2:3].squeeze(3)
        eng.tensor_tensor(out=dout, in0=dout,
                          in1=m9(IN)[:, PAD:W, 2:6:3], op=Alu.add)

    def compose_red(OUT, XAP, YAP, G, PRt):
        """OUT[g] = X[g] o Y[g] on DVE: 2 strided mults + one reduce.
        XAP/YAP: [p, g, 9] homogeneous map views (X may be PSUM)."""
        PRv = PRt.rearrange("p (r g i k) -> p r g i k", r=2, g=G, i=3, k=3)
        Yki = (YAP.rearrange("p g (k i) -> p g k i", k=3, i=3)
               .rearrange("p g k i -> p g i k"))
        for r in (0, 1):
            Xr = (XAP[:, :, 3 * r:3 * r + 3]
                  .unsqueeze(2).broadcast_to((R, G, 3, 3)))
            V.tensor_tensor(out=PRv[:, r], in0=Xr, in1=Yki, op=Alu.mult)
        OUTg = OUT.rearrange("p (g r i) -> p g r i", g=G, r=3, i=3)
        PRred = PRt.rearrange("p (x k) -> p x k", k=3)
        V.tensor_reduce(out=(OUT.rearrange("p (g r i) -> p r g i", g=G, r=3)
                             [:, 0:2]),
                        in_=PRred, axis=Ax.X, op=Alu.add)

    # ---- mini end-column ladder on DVE (feeds the cross-row early) ----
    PRe4 = T("PRe4", [R, 2 * 8 * 9])
    PRe8 = T("PRe8", [R, 2 * 4 * 9])
    PRe16 = T("PRe16", [R, 2 * 2 * 9])
    compose_red(E4, m9(M2)[:, PAD + 3:W:4], m9(M2)[:, PAD + 1:W:4], 8, PRe4)
    E4g = E4.rearrange("p (g x) -> p g x", g=8)
    compose_red(E8, E4g[:, 1:8:2], E4g[:, 0:8:2], 4, PRe8)
    E8g = E8.rearrange("p (g x) -> p g x", g=4)
    compose_red(E16, E8g[:, 1:4:2], E8g[:, 0:4:2], 2, PRe16)

    # ---- burst 1: shift [span16@t15 | span16@t31] by 0..3 ----
    ps1 = psum_pool.tile([R, 4 * 18], F, name="ps1", tag="ps1")
    for g, n in enumerate((0, 1, 2, 3)):
        nc.tensor.matmul(ps1[:, 18 * g:18 * g + 18], sh[n], E16,
                         start=True, stop=True)

    # ---- M4 level on DVE: fills the burst-1 PE/PSUM latency gap ----
    PPd = T("PPd", [R, 2 * L * 6])
    compose_full(V, M4, M2, 2, PPd)

    V.tensor_copy(out=KS1, in_=ps1)
    KS1g = KS1.rearrange("p (g b x) -> p g b x", g=4, b=2)
    PRh = T("PRh", [R, 2 * 4 * 9])
    compose_red(KH, KS1g[:, :, 1], KS1g[:, :, 0], 4, PRh)
    KHx = KH.rearrange("p (g x) -> p g x", g=4)
    PRt = T("PRt", [R, 2 * 2 * 9])
    compose_red(TF, KHx[:, 0:4:2], KHx[:, 1:4:2], 2, PRt)
    PRk = T("PRk", [R, 2 * 9])
    TFx = TF.rearrange("p (g x) -> p g x", g=2)
    compose_red(K4, TFx[:, 0:1], TFx[:, 1:2], 1, PRk)
    # ---- burst 2: K4 shifted by 1, 5, 9, 13 ----
    ps2 = psum_pool.tile([R, 4 * 9], F, name="ps2", tag="ps2")
    for g, n in enumerate((1, 5, 9, 13)):
        nc.tensor.matmul(ps2[:, 9 * g:9 * g + 9], sh[n], K4,
                         start=True, stop=True)
    V.tensor_copy(out=KS2, in_=ps2)
    KS2g = KS2.rearrange("p (g x) -> p g x", g=4)
    PRu = T("PRu", [R, 2 * 2 * 9])
    compose_red(TT, KS2g[:, 0:4:2], KS2g[:, 1:4:2], 2, PRu)
    PRv2 = T("PRv2", [R, 2 * 9])
    TTx = TT.rearrange("p (g x) -> p g x", g=2)
    compose_red(K16, TTx[:, 0:1], TTx[:, 1:2], 1, PRv2)
    # rho_p = K16.A_p @ zi + K16.D_p (state at start of row p)
    K16x = K16.rearrange("p (r c) -> p r c", r=3)
    rho_t = T("rho_t", [R, 2])
    V.scalar_tensor_tensor(out=rho_t, in0=K16x[:, 0:2, 1], scalar=zi2_ap,
                           in1=K16x[:, 0:2, 2], op0=Alu.mult, op1=Alu.add)
    rho = T("rho", [R, 2])
    V.scalar_tensor_tensor(out=rho, in0=K16x[:, 0:2, 0], scalar=zi1_ap,
                           in1=rho_t, op0=Alu.mult, op1=Alu.add)

    # ---- full-width ladder tail on Pool (concurrent with DVE tree) ----
    PPp = T("PPp", [R, 2 * L * 6])
    compose_full(GP, M8, M4, 4, PPp)
    compose_full(GP, M16, M8, 8, PPp)

    # ---- apply ----
    # y[t] = b0d[t] + row0(prefix[t-1]) . (rho1, rho2, 1).
    # t in [0, 17): prefix[t-1] = M16 row0 at map col PAD-1+t (identity pad
    # at t=0). t in [17, 32): FR[j] = row0(M16[16+j] o M16[j]), j = t-17.
    M16f = m9(M16)
    PRf = T("PRf", [R, 15 * 9])
    PRfv = PRf.rearrange("p (j i k) -> p j i k", i=3, k=3)
    V.tensor_tensor(
        out=PRfv,
        in0=M16f[:, PAD + 16:W - 1, 0:3].unsqueeze(2)
        .broadcast_to((R, 15, 3, 3)),
        in1=(M16.rearrange("p (t k i) -> p t k i", k=3, i=3)
             [:, PAD:PAD + 15].rearrange("p t k i -> p t i k")),
        op=Alu.mult)
    FR = T("FR", [R, 15 * 3])
    V.tensor_reduce(out=FR.rearrange("p (j i) -> p j i", i=3),
                    in_=PRf.rearrange("p (x k) -> p x k", k=3),
                    axis=Ax.X, op=Alu.add)
    FRv = FR.rearrange("p (j i) -> p j i", i=3)
    yA = T("yA", [R, L])
    lv = m9(M16)[:, PAD - 1:PAD + 16]
    V.scalar_tensor_tensor(out=yA[:, 0:17], in0=lv[:, :, 1:2].squeeze(2),
                           scalar=rho[:, 1:2], in1=lv[:, :, 2:3].squeeze(2),
                           op0=Alu.mult, op1=Alu.add)
    V.scalar_tensor_tensor(out=yA[:, 17:L], in0=FRv[:, :, 1:2].squeeze(2),
                           scalar=rho[:, 1:2], in1=FRv[:, :, 2:3].squeeze(2),
                           op0=Alu.mult, op1=Alu.add)
    y1 = T("y1", [R, L])
    V.scalar_tensor_tensor(out=y1[:, 0:17], in0=lv[:, :, 0:1].squeeze(2),
                           scalar=rho[:, 0:1], in1=yA[:, 0:17],
                           op0=Alu.mult, op1=Alu.add)
    V.scalar_tensor_tensor(out=y1[:, 17:L], in0=FRv[:, :, 0:1].squeeze(2),
                           scalar=rho[:, 0:1], in1=yA[:, 17:L],
                           op0=Alu.mult, op1=Alu.add)
    y = T("y", [R, L])
    V.tensor_add(y, b0d, y1)
    wet = T("wet", [R, L])
    S.activation(wet[96:128, :], y[96:128, :], Act.Tanh)
    nc.sync.dma_start(out=y_out, in_=wet[96:128, :])


def _build():
    import concourse.bacc as bacc
    import concourse.mybir as mybir
    from concourse.tile import TileContext

    F = mybir.dt.float32
    nc = bacc.Bacc("TRN2", target_bir_lowering=False, debug=False,
                   enable_asserts=True, num_devices=8)
    in_all = nc.dram_tensor("in_all", [R, IC], F, kind="ExternalInput").ap()
    y_out = nc.dram_tensor("wet_out", [32, L], F, kind="ExternalOutput").ap()
    with TileContext(nc) as tc:
        with tc.tile_pool(name="p", bufs=1) as pool, \
             tc.tile_pool(name="ps", bufs=1, space="PSUM") as psum_pool:
            _emit(nc, tc, pool, psum_pool, in_all, y_out)
    nc.compile()
    return nc


def _host_inputs(midi_f0_0to1, alpha_0to1, w_mod_sig, q_mod_sig, phase, zi):
    """Per-core input pack [R, IC]: scalar cols (rosc, pbase, zi1, zi2),
    w rows, q rows, env rows, zero pad. Chunk c covers global samples
    [c*1024-3072, c*1024+1024); negative-t rows get zero w/q/env, which
    pins the filter input (and state) to zero until t=0."""
    f32 = np.float32
    alpha = np.float64(f32(alpha_0to1.reshape(-1)[0]) * f32(3.0 - 0.2) + f32(0.2))
    midi = f32(np.round(f32(midi_f0_0to1.reshape(-1)[0]) * f32(60.0 - 30.0) + f32(30.0)))
    f0 = f32(f32(440.0) * f32(2.0) ** f32((midi - f32(69.0)) / f32(12.0)))
    r64 = np.float64(f0) / 48000.0
    p64 = np.float64(phase.reshape(-1)[0]) / (2.0 * np.pi)
    wfull = w_mod_sig.reshape(-1)[:A].astype(f32)
    qfull = q_mod_sig.reshape(-1)[:A].astype(f32)
    tg = np.arange(A, dtype=np.float64)
    envfull = (np.clip(1.0 - tg / 6000.0, 0.0, 1.0) ** alpha).astype(f32)
    maps = []
    for c in range(8):
        cs = c * PAY - (CH - PAY)
        rows = np.arange(R, dtype=np.float64)
        base = np.mod(p64 + r64 * (cs + L * rows), 1.0)
        allin = np.zeros((R, IC), f32)
        allin[:, 0] = f32(r64)
        allin[:, 1] = base.astype(f32)
        allin[:, 2] = f32(zi.reshape(-1)[0])
        allin[:, 3] = f32(zi.reshape(-1)[1])
        wp = np.zeros(CH, f32)
        qp = np.zeros(CH, f32)
        ep = np.zeros(CH, f32)
        lo = max(0, -cs)
        wp[lo:] = wfull[cs + lo:cs + CH]
        qp[lo:] = qfull[cs + lo:cs + CH]
        ep[lo:] = envfull[cs + lo:cs + CH]
        allin[:, SC:SC + L] = wp.reshape(R, L)
        allin[:, SC + L:SC + 2 * L] = qp.reshape(R, L)
        allin[:, SC + 2 * L:SC + 3 * L] = ep.reshape(R, L)
        maps.append({"in_all": allin})
    return maps


def kernel(x, midi_f0_0to1, alpha_0to1, w_mod_sig, q_mod_sig, phase, zi,
           _trace=False):
    from concourse import bass_utils

    midi_f0_0to1 = np.asarray(midi_f0_0to1)
    alpha_0to1 = np.asarray(alpha_0to1)
    w_mod_sig = np.asarray(w_mod_sig)
    q_mod_sig = np.asarray(q_mod_sig)
    phase = np.asarray(phase)
    zi = np.asarray(zi)
    if "nc" not in _cache:
        _cache["nc"] = _build()
    nc = _cache["nc"]
    in_maps = _host_inputs(midi_f0_0to1, alpha_0to1, w_mod_sig, q_mod_sig,
                           phase, zi)
    res = bass_utils.run_bass_kernel_spmd(
        nc, in_maps, core_ids=list(range(8)), trace=_trace)
    _cache["last_result"] = res
    out = np.zeros((1, N), np.float32)
    for c in range(8):
        out[0, c * PAY:(c + 1) * PAY] = res.results[c]["wet_out"].reshape(-1)
    return out


# revision 15
# speedup vs baseline: 1.2890x; 1.2890x over previous
"""AcidSynth Trainium2 kernel (v4).

Only the first 8192 output samples are nonzero (env dies at t=6000; the
dissipative biquad state underflows to fp32 zero soon after). 8 cores
each compute a 4096-sample chunk (3072 warmup + 1024 payload at rows
96:128 of a [128 x 32] layout); the rest of the 524288-sample output is
assembled as zeros on host.

Per-sample affine state maps are HOMOGENEOUS 3x3 blocks (9 slots,
row-major, constant bottom row (0,0,1) written once at setup), so a map
compose Z = X o Y is plainly Z = X @ Y restricted to rows 0-1:
  * latency-critical composes (DVE) = 2 strided mults + one
    tensor_reduce over k — dependency depth 2, no separate d-fix;
  * throughput composes (Pool full-width ladder) = 2 mults (k in {0,1})
    + pair-add + d-fix, which is cheaper there.

Cross-row state uses the 16-row (512-sample) windowed composition with
the same accuracy envelope as the validated baseline. Key scheduling:
  * dummy Sin activation at the ACT queue head prefetches the 1283ns
    trig table during the input DMA;
  * env (pure function of scalar alpha and t) host-computed; oscillator
    (phase recurrence) on device;
  * a mini end-column ladder (E4/E8/E16: span-4/8/16 composites at the
    16/8/4... end columns only) races ahead on DVE, so the PE shift
    bursts + 16-row window tree run ~4us before the full ladder is done;
  * the full-width ladder: M4 on DVE inside the burst-1 PE latency gap,
    M8/M16 on Pool concurrently with the DVE window tree;
  * no identity fixups in shift bursts (only rows 96:128 are output);
  * apply is split: y[0:17] reads M16's row0 directly (prefix spans
    <= 16 need no extra compose; col PAD-1 is the identity pad), y[17:32]
    reads a 15-column row0-compose FR = row0(M16[16+j] o M16[j]).
"""

import numpy as np

R = 128          # rows (SBUF partitions)
L = 32           # samples per row
PAD = 16         # identity-map pad columns for in-row KS shifts
W = L + PAD
CH = R * L       # per-core chunk = 4096
PAY = 1024       # payload samples per core
A = 8192         # active window
N = 524288
SC = 8           # scalar columns in the input pack
IC = SC + 3 * L + 28   # input cols padded to 132 (528B rows: full-rate DMA)

_cache = {}


def _emit(nc, tc, pool, psum_pool, in_all, y_out):
    import concourse.mybir as mybir

    F = mybir.dt.float32
    I32 = mybir.dt.int32
    Alu = mybir.AluOpType
    Act = mybir.ActivationFunctionType
    Ax = mybir.AxisListType
    V = nc.vector
    S = nc.scalar
    GP = nc.gpsimd

    def T(name, shape, dtype=F):
        return pool.tile(shape, dtype, name=name, tag=name)

    # ---------------- input DMA (single, posted first) ----------------
    allin = T("allin", [R, IC])
    nc.sync.dma_start(out=allin, in_=in_all)
    sc = allin[:, 0:SC]
    wv = allin[:, SC:SC + L]
    qv = allin[:, SC + L:SC + 2 * L]
    env = allin[:, SC + 2 * L:SC + 3 * L]
    rosc_ap = sc[:, 0:1]
    pbase_ap = sc[:, 1:2]
    zi1_ap = sc[:, 2:3]
    zi2_ap = sc[:, 3:4]

    # ---------------- pre-DMA setup ----------------
    M2 = T("M2", [R, W * 9])
    M4 = T("M4", [R, W * 9])
    M8 = T("M8", [R, W * 9])
    M16 = T("M16", [R, W * 9])
    NAC = T("NAC", [R, L * 4])       # per-sample (na1, na2, c1, c2)
    KS1 = T("KS1", [R, 4 * 18])      # burst-1 shifted 18-col packs (SBUF)
    KS2 = T("KS2", [R, 4 * 9])       # burst-2 shifted K4 maps (SBUF)

    def m9(M):
        return M.rearrange("p (t x) -> p t x", x=9)

    bcs = T("bcs", [R, 2])
    SCLW = float(np.float32(2.0 * np.pi * 7900.0 / 48000.0))
    BS = float(np.float32(2.0 * np.pi * 100.0 / 48000.0))
    BC = float(np.float32(BS + np.pi / 2))
    V.memset(bcs[:, 0:1], BC)
    V.memset(bcs[:, 1:2], BS)
    # Dummy Sin with no DMA dependency: hoists the trig table load to the
    # ACT queue head so it overlaps the input DMA. Its output cell is
    # overwritten by the burst-1 PSUM copy before any read.
    S.activation(KS1[:, 0:1], bcs[:, 0:1], Act.Sin)

    # Identity pads only where a compose actually reaches back: level d
    # reads d pad columns. Row2 (0,0,1) constants only where a
    # reduce-compose reads the tile as its Y operand.
    def id_pads(eng, M, npad):
        eng.memset(M[:, (PAD - npad) * 9:PAD * 9], 0.0)
        eng.memset(m9(M)[:, PAD - npad:PAD, 0:1], 1.0)
        eng.memset(m9(M)[:, PAD - npad:PAD, 4:5], 1.0)

    id_pads(V, M2, 2)
    id_pads(GP, M4, 4)
    id_pads(GP, M8, 8)
    id_pads(V, M16, 1)
    # M2 is the Y operand of E4 (odd cols); M16 of FR (cols PAD..PAD+14)
    # and of the left apply (col PAD-1).
    V.memset(m9(M2)[:, PAD:W, 6:8], 0.0)
    V.memset(m9(M2)[:, PAD:W, 8:9], 1.0)
    V.memset(m9(M16)[:, PAD - 1:PAD + 15, 6:8], 0.0)
    V.memset(m9(M16)[:, PAD - 1:PAD + 15, 8:9], 1.0)
    V.memset(M2[:, PAD * 9 + 1:PAD * 9 + 2], 1.0)   # t=0: a01 = 1
    V.memset(M2[:, PAD * 9 + 4:PAD * 9 + 5], 0.0)   # t=0: a11 = 0

    def row2_const(eng, Mt):
        v = Mt.rearrange("p (g x) -> p g x", x=9)
        eng.memset(v[:, :, 6:8], 0.0)
        eng.memset(v[:, :, 8:9], 1.0)

    E4 = T("E4", [R, 8 * 9])     # span-4 composites at t = 4j+3
    E8 = T("E8", [R, 4 * 9])     # span-8 at t = 8j+7
    E16 = T("E16", [R, 2 * 9])   # span-16 at t = 15, 31
    KH = T("KH", [R, 4 * 9])     # H, Hs1, Hs2, Hs3 (H = span-32 row map)
    TF = T("TF", [R, 2 * 9])
    K4 = T("K4", [R, 9])
    TT = T("TT", [R, 2 * 9])
    K16 = T("K16", [R, 9])
    for eng, Mt in ((V, E4), (V, E8), (V, E16), (GP, KH), (GP, TF),
                    (GP, K4), (GP, TT)):
        row2_const(eng, Mt)

    ji = T("ji", [R, L], I32)
    GP.iota(ji, pattern=[[1, L]], base=0, channel_multiplier=0)
    jf = T("jf", [R, L])
    V.tensor_copy(out=jf, in_=ji)
    ii = T("ii", [R, R], I32)        # ii[c, j] = j - c
    GP.iota(ii, pattern=[[1, R]], base=0, channel_multiplier=-1)
    iif = T("iif", [R, R])
    V.tensor_copy(out=iif, in_=ii)
    sh = {}
    for n, eng in ((0, V), (1, V), (2, V), (3, V), (5, GP), (9, GP), (13, GP)):
        m = T("sh%d" % n, [R, R])
        eng.tensor_scalar(m, iif, float(n), None, Alu.is_equal)
        sh[n] = m

    # ---------------- coefficient chain (post-DMA) ----------------
    cw = T("cw", [R, L])
    S.activation(cw, wv, Act.Sin, bias=bcs[:, 0:1], scale=SCLW)
    sw = T("sw", [R, L])
    S.activation(sw, wv, Act.Sin, bias=bcs[:, 1:2], scale=SCLW)
    q2 = T("q2", [R, L])
    V.tensor_scalar(q2, qv, float(np.float32(2.0 * (8.0 - 0.7071))),
                    float(np.float32(2.0 * 0.7071)), Alu.mult, Alu.add)
    rq = T("rq", [R, L])
    V.reciprocal(rq, q2)
    # oscillator (independent of the w/q chain)
    uph = T("uph", [R, L])
    V.tensor_scalar(uph, jf, rosc_ap, pbase_ap, Alu.mult, Alu.add)
    ge1 = T("ge1", [R, L])
    V.tensor_scalar(ge1, uph, 1.0, None, Alu.is_ge)
    ph = T("ph", [R, L])
    V.tensor_tensor(out=ph, in0=uph, in1=ge1, op=Alu.subtract)
    dp = T("dp", [R, L])
    V.tensor_scalar(dp, ph, 0.5, 0.5, Alu.is_lt, Alu.subtract)
    dry = T("dry", [R, L])
    V.tensor_mul(dry, dp, env)

    af = T("af", [R, L])
    V.tensor_mul(af, sw, rq)
    a0 = T("a0", [R, L])
    V.tensor_scalar_add(a0, af, 1.0)
    r0 = T("r0", [R, L])
    V.reciprocal(r0, a0)
    cwh = T("cwh", [R, L])           # (1-cw)/2
    V.tensor_scalar(cwh, cw, -0.5, 0.5, Alu.mult, Alu.add)
    cd = T("cd", [R, L])             # (1-cw)/2 * dry
    V.tensor_mul(cd, cwh, dry)

    NAC4 = NAC.rearrange("p (t s) -> p t s", s=4)
    na1v = NAC4[:, :, 0:1].squeeze(2)
    na2v = NAC4[:, :, 1:2].squeeze(2)
    c1v = NAC4[:, :, 2:3].squeeze(2)
    c2v = NAC4[:, :, 3:4].squeeze(2)
    V.scalar_tensor_tensor(out=na1v, in0=cw, scalar=2.0, in1=r0,
                           op0=Alu.mult, op1=Alu.mult)
    # na2 = (af-1)/a0 = 1 - 2*r0
    V.tensor_scalar(na2v, r0, -2.0, 1.0, Alu.mult, Alu.add)
    b0d = T("b0d", [R, L])           # b0*dry
    V.tensor_mul(b0d, cd, r0)
    V.scalar_tensor_tensor(out=c1v, in0=na1v, scalar=2.0, in1=b0d,
                           op0=Alu.add, op1=Alu.mult)
    V.scalar_tensor_tensor(out=c2v, in0=na2v, scalar=1.0, in1=b0d,
                           op0=Alu.add, op1=Alu.mult)

    # ---------------- span-2 construct into M2 ----------------
    # Z[t]: a00 = na1_t*na1' + na2';  a01 = na1_t
    #       d1  = na1_t*c1'  + c2' + c1_t
    #       a10 = na2_t*na1';         a11 = na2_t
    #       d2  = na2_t*c1'  + c2_t           (x' = x_{t-1})
    M2trg = M2.rearrange("p (t r g) -> p t r g", r=3, g=3)
    Lm = L - 1
    GP.tensor_copy(out=M2trg[:, PAD + 1:W, 0:2, 1:2].squeeze(3),
                   in_=NAC4[:, 1:L, 0:2])
    GP.tensor_copy(out=M2trg[:, PAD:PAD + 1, 0:2, 0:1].squeeze(3).squeeze(1),
                   in_=NAC4[:, 0:1, 0:2].squeeze(1))
    GP.tensor_copy(out=M2trg[:, PAD:PAD + 1, 0:2, 2:3].squeeze(3).squeeze(1),
                   in_=NAC4[:, 0:1, 2:4].squeeze(1))
    pm_out = M2trg[:, PAD + 1:W, 0:2, 0:3:2]
    V.tensor_tensor(
        out=pm_out,
        in0=NAC4[:, 1:L, 0:2].unsqueeze(3).broadcast_to((R, Lm, 2, 2)),
        in1=NAC4[:, 0:Lm, 0:3:2].unsqueeze(2).broadcast_to((R, Lm, 2, 2)),
        op=Alu.mult)
    aa_out = M2trg[:, PAD + 1:W, 0:1, 0:3:2].squeeze(2)   # {a00, d1}
    V.tensor_tensor(out=aa_out, in0=aa_out, in1=NAC4[:, 0:Lm, 1:4:2],
                    op=Alu.add)
    ab_out = M2trg[:, PAD + 1:W, 0:2, 2:3].squeeze(3)     # {d1, d2}
    V.tensor_tensor(out=ab_out, in0=ab_out, in1=NAC4[:, 1:L, 2:4],
                    op=Alu.add)

    # ---------------- composes ----------------
    def compose_full(eng, OUT, IN, d, PPt):
        """OUT[t] = IN[t] o IN[t-d], all columns (2 mults + add + fix)."""
        PPv = PPt.rearrange("p (r t i k) -> p r t i k", r=2, t=L, i=3, k=2)
        INx = m9(IN)
        Yv = (IN.rearrange("p (t k i) -> p t k i", k=3, i=3)
              [:, PAD - d:W - d, 0:2].rearrange("p t k i -> p t i k"))
        for r in (0, 1):
            Xr = (INx[:, PAD:W, 3 * r:3 * r + 2]
                  .unsqueeze(2).broadcast_to((R, L, 3, 2)))
            eng.tensor_tensor(out=PPv[:, r], in0=Xr, in1=Yv, op=Alu.mult)
        OUTtrg = OUT.rearrange("p (t r g) -> p t r g", r=3, g=3)
        PPtr = PPt.rearrange("p (r t i k) -> p t r i k", r=2, t=L, i=3, k=2)
        eng.tensor_tensor(out=OUTtrg[:, PAD:W, 0:2], in0=PPtr[:, :, :, :, 0],
                          in1=PPtr[:, :, :, :, 1], op=Alu.add)
        dout = OUTtrg[:, PAD:W, 0:2, 2:3].squeeze(3)
        eng.tensor_tensor(out=dout, in0=dout,
                          in1=m9(IN)[:, PAD:W, 2:6:3], op=Alu.add)

    def compose_red(OUT, XAP, YAP, G, PRt):
        """OUT[g] = X[g] o Y[g] on DVE: 2 strided mults + one reduce.
        XAP/YAP: [p, g, 9] homogeneous map views (X may be PSUM)."""
        PRv = PRt.rearrange("p (r g i k) -> p r g i k", r=2, g=G, i=3, k=3)
        Yki = (YAP.rearrange("p g (k i) -> p g k i", k=3, i=3)
               .rearrange("p g k i -> p g i k"))
        for r in (0, 1):
            Xr = (XAP[:, :, 3 * r:3 * r + 3]
                  .unsqueeze(2).broadcast_to((R, G, 3, 3)))
            V.tensor_tensor(out=PRv[:, r], in0=Xr, in1=Yki, op=Alu.mult)
        OUTg = OUT.rearrange("p (g r i) -> p g r i", g=G, r=3, i=3)
        PRred = PRt.rearrange("p (x k) -> p x k", k=3)
        V.tensor_reduce(out=(OUT.rearrange("p (g r i) -> p r g i", g=G, r=3)
                             [:, 0:2]),
                        in_=PRred, axis=Ax.X, op=Alu.add)

    # ---- mini end-column ladder on DVE (feeds the cross-row early) ----
    PRe4 = T("PRe4", [R, 2 * 8 * 9])
    PRe8 = T("PRe8", [R, 2 * 4 * 9])
    PRe16 = T("PRe16", [R, 2 * 2 * 9])
    compose_red(E4, m9(M2)[:, PAD + 3:W:4], m9(M2)[:, PAD + 1:W:4], 8, PRe4)
    E4g = E4.rearrange("p (g x) -> p g x", g=8)
    compose_red(E8, E4g[:, 1:8:2], E4g[:, 0:8:2], 4, PRe8)
    E8g = E8.rearrange("p (g x) -> p g x", g=4)
    compose_red(E16, E8g[:, 1:4:2], E8g[:, 0:4:2], 2, PRe16)

    # ---- burst 1: shift [span16@t15 | span16@t31] by 0..3 ----
    ps1 = psum_pool.tile([R, 4 * 18], F, name="ps1", tag="ps1")
    for g, n in enumerate((0, 1, 2, 3)):
        nc.tensor.matmul(ps1[:, 18 * g:18 * g + 18], sh[n], E16,
                         start=True, stop=True)
    V.tensor_copy(out=KS1, in_=ps1)
    KS1g = KS1.rearrange("p (g b x) -> p g b x", g=4, b=2)
    PRh = T("PRh", [R, 2 * 4 * 9])
    compose_red(KH, KS1g[:, :, 1], KS1g[:, :, 0], 4, PRh)
    KHx = KH.rearrange("p (g x) -> p g x", g=4)
    PRt = T("PRt", [R, 2 * 2 * 9])
    compose_red(TF, KHx[:, 0:4:2], KHx[:, 1:4:2], 2, PRt)
    PRk = T("PRk", [R, 2 * 9])
    TFx = TF.rearrange("p (g x) -> p g x", g=2)
    compose_red(K4, TFx[:, 0:1], TFx[:, 1:2], 1, PRk)
    # ---- burst 2: K4 shifted by 1, 5, 9, 13 ----
    ps2 = psum_pool.tile([R, 4 * 9], F, name="ps2", tag="ps2")
    for g, n in enumerate((1, 5, 9, 13)):
        nc.tensor.matmul(ps2[:, 9 * g:9 * g + 9], sh[n], K4,
                         start=True, stop=True)
    V.tensor_copy(out=KS2, in_=ps2)
    KS2g = KS2.rearrange("p (g x) -> p g x", g=4)
    PRu = T("PRu", [R, 2 * 2 * 9])
    compose_red(TT, KS2g[:, 0:4:2], KS2g[:, 1:4:2], 2, PRu)
    PRv2 = T("PRv2", [R, 2 * 9])
    TTx = TT.rearrange("p (g x) -> p g x", g=2)
    compose_red(K16, TTx[:, 0:1], TTx[:, 1:2], 1, PRv2)
    # rho_p = K16.A_p @ zi + K16.D_p (state at start of row p)
    K16x = K16.rearrange("p (r c) -> p r c", r=3)
    rho_t = T("rho_t", [R, 2])
    V.scalar_tensor_tensor(out=rho_t, in0=K16x[:, 0:2, 1], scalar=zi2_ap,
                           in1=K16x[:, 0:2, 2], op0=Alu.mult, op1=Alu.add)
    rho = T("rho", [R, 2])
    V.scalar_tensor_tensor(out=rho, in0=K16x[:, 0:2, 0], scalar=zi1_ap,
                           in1=rho_t, op0=Alu.mult, op1=Alu.add)

    # ---- full-width ladder on Pool (concurrent with the DVE tree) ----
    PPp = T("PPp", [R, 2 * L * 6])
    compose_full(GP, M4, M2, 2, PPp)
    compose_full(GP, M8, M4, 4, PPp)
    compose_full(GP, M16, M8, 8, PPp)

    # ---- apply ----
    # y[t] = b0d[t] + row0(prefix[t-1]) . (rho1, rho2, 1).
    # t in [0, 17): prefix[t-1] = M16 row0 at map col PAD-1+t (identity pad
    # at t=0). t in [17, 32): FR[j] = row0(M16[16+j] o M16[j]), j = t-17.
    # tile_wait_until: keep these late consumers (which wait on Pool's M16
    # and on rho) out of the DVE queue's middle — without it the scheduler
    # interleaves them into the window tree and head-of-line blocks it.
    M16f = m9(M16)
    PRf = T("PRf", [R, 15 * 9])
    PRfv = PRf.rearrange("p (j i k) -> p j i k", i=3, k=3)
    with tc.tile_wait_until(0.012):
        V.tensor_tensor(
            out=PRfv,
            in0=M16f[:, PAD + 16:W - 1, 0:3].unsqueeze(2)
            .broadcast_to((R, 15, 3, 3)),
            in1=(M16.rearrange("p (t k i) -> p t k i", k=3, i=3)
                 [:, PAD:PAD + 15].rearrange("p t k i -> p t i k")),
            op=Alu.mult)
        FR = T("FR", [R, 15 * 3])
        V.tensor_reduce(out=FR.rearrange("p (j i) -> p j i", i=3),
                        in_=PRf.rearrange("p (x k) -> p x k", k=3),
                        axis=Ax.X, op=Alu.add)
        FRv = FR.rearrange("p (j i) -> p j i", i=3)
        yA = T("yA", [R, L])
        lv = m9(M16)[:, PAD - 1:PAD + 16]
        V.scalar_tensor_tensor(out=yA[:, 0:17], in0=lv[:, :, 1:2].squeeze(2),
                               scalar=rho[:, 1:2],
                               in1=lv[:, :, 2:3].squeeze(2),
                               op0=Alu.mult, op1=Alu.add)
        V.scalar_tensor_tensor(out=yA[:, 17:L], in0=FRv[:, :, 1:2].squeeze(2),
                               scalar=rho[:, 1:2],
                               in1=FRv[:, :, 2:3].squeeze(2),
                               op0=Alu.mult, op1=Alu.add)
        y1 = T("y1", [R, L])
        V.scalar_tensor_tensor(out=y1[:, 0:17], in0=lv[:, :, 0:1].squeeze(2),
                               scalar=rho[:, 0:1], in1=yA[:, 0:17],
                               op0=Alu.mult, op1=Alu.add)
        V.scalar_tensor_tensor(out=y1[:, 17:L], in0=FRv[:, :, 0:1].squeeze(2),
                               scalar=rho[:, 0:1], in1=yA[:, 17:L],
                               op0=Alu.mult, op1=Alu.add)
        y = T("y", [R, L])
        V.tensor_add(y, b0d, y1)
    wet = T("wet", [R, L])
    S.activation(wet[96:128, :], y[96:128, :], Act.Tanh)
    nc.sync.dma_start(out=y_out, in_=wet[96:128, :])


def _build():
    import concourse.bacc as bacc
    import concourse.mybir as mybir
    from concourse.tile import TileContext

    F = mybir.dt.float32
    nc = bacc.Bacc("TRN2", target_bir_lowering=False, debug=False,
                   enable_asserts=True, num_devices=8)
    in_all = nc.dram_tensor("in_all", [R, IC], F, kind="ExternalInput").ap()
    y_out = nc.dram_tensor("wet_out", [32, L], F, kind="ExternalOutput").ap()
    with TileContext(nc) as tc:
        with tc.tile_pool(name="p", bufs=1) as pool, \
             tc.tile_pool(name="ps", bufs=1, space="PSUM") as psum_pool:
            _emit(nc, tc, pool, psum_pool, in_all, y_out)
    nc.compile()
    return nc


def _host_inputs(midi_f0_0to1, alpha_0to1, w_mod_sig, q_mod_sig, phase, zi):
    """Per-core input pack [R, IC]: scalar cols (rosc, pbase, zi1, zi2),
    w rows, q rows, env rows, zero pad. Chunk c covers global samples
    [c*1024-3072, c*1024+1024); negative-t rows get zero w/q/env, which
    pins the filter input (and state) to zero until t=0."""
    f32 = np.float32
    alpha = np.float64(f32(alpha_0to1.reshape(-1)[0]) * f32(3.0 - 0.2) + f32(0.2))
    midi = f32(np.round(f32(midi_f0_0to1.reshape(-1)[0]) * f32(60.0 - 30.0) + f32(30.0)))
    f0 = f32(f32(440.0) * f32(2.0) ** f32((midi - f32(69.0)) / f32(12.0)))
    r64 = np.float64(f0) / 48000.0
    p64 = np.float64(phase.reshape(-1)[0]) / (2.0 * np.pi)
    wfull = w_mod_sig.reshape(-1)[:A].astype(f32)
    qfull = q_mod_sig.reshape(-1)[:A].astype(f32)
    tg = np.arange(A, dtype=np.float64)
    envfull = (np.clip(1.0 - tg / 6000.0, 0.0, 1.0) ** alpha).astype(f32)
    maps = []
    for c in range(8):
        cs = c * PAY - (CH - PAY)
        rows = np.arange(R, dtype=np.float64)
        base = np.mod(p64 + r64 * (cs + L * rows), 1.0)
        allin = np.zeros((R, IC), f32)
        allin[:, 0] = f32(r64)
        allin[:, 1] = base.astype(f32)
        allin[:, 2] = f32(zi.reshape(-1)[0])
        allin[:, 3] = f32(zi.reshape(-1)[1])
        wp = np.zeros(CH, f32)
        qp = np.zeros(CH, f32)
        ep = np.zeros(CH, f32)
        lo = max(0, -cs)
        wp[lo:] = wfull[cs + lo:cs + CH]
        qp[lo:] = qfull[cs + lo:cs + CH]
        ep[lo:] = envfull[cs + lo:cs + CH]
        allin[:, SC:SC + L] = wp.reshape(R, L)
        allin[:, SC + L:SC + 2 * L] = qp.reshape(R, L)
        allin[:, SC + 2 * L:SC + 3 * L] = ep.reshape(R, L)
        maps.append({"in_all": allin})
    return maps


def kernel(x, midi_f0_0to1, alpha_0to1, w_mod_sig, q_mod_sig, phase, zi,
           _trace=False):
    from concourse import bass_utils

    midi_f0_0to1 = np.asarray(midi_f0_0to1)
    alpha_0to1 = np.asarray(alpha_0to1)
    w_mod_sig = np.asarray(w_mod_sig)
    q_mod_sig = np.asarray(q_mod_sig)
    phase = np.asarray(phase)
    zi = np.asarray(zi)
    if "nc" not in _cache:
        _cache["nc"] = _build()
    nc = _cache["nc"]
    in_maps = _host_inputs(midi_f0_0to1, alpha_0to1, w_mod_sig, q_mod_sig,
                           phase, zi)
    res = bass_utils.run_bass_kernel_spmd(
        nc, in_maps, core_ids=list(range(8)), trace=_trace)
    _cache["last_result"] = res
    out = np.zeros((1, N), np.float32)
    for c in range(8):
        out[0, c * PAY:(c + 1) * PAY] = res.results[c]["wet_out"].reshape(-1)
    return out
